# revision 10
# baseline (speedup 1.0000x reference)
"""Trainium2 Bass kernel for nn_BaselineSpanScorer (span-pair MLP scorer).

reference:
    xs        [32, 512, 1024] f32
    spans     [65536, 2] int   (begin/end token index within sequence)
    batch_ids [65536] int
    W1 [2048, 150], b1 [150], W2 [150, 17], b2 [17]
    out[n] = relu(concat(xs[b, s0], xs[b, s1]) @ W1 + b1) @ W2 + b2

Strategy (8 NeuronCores, data parallel with host routing):
  - Shard xs by batch: core c owns batches [4c, 4c+4) = 2048 token rows.
  - Route each span to the core owning its batch (original order kept).
  - Algebraic factorization: per token t precompute
        A[t] = xs[t] @ W1[:1024],  G[t] = xs[t] @ W1[1024:]
    so pre[n] = A[i0_n] + G[i1_n] + b1 (avg token reuse ~8x cuts matmul
    work 4x vs direct span scoring).
  - Stage 1: A|G rows via TensorE fp16 into an SBUF-resident table
    [128, 32 stripes x 512B]: token t's A row lands at partition t%128,
    stripe t//128; G rows at stripes 16..31. PSUM->SBUF copies write the
    stripes directly - no DRAM round trip.
  - Stage 2 per 1024-span chunk: ONE SBUF-source transpose dma_gather of
    2048 rows (A rows i0, G rows 2048+i1). Features land on partitions
    (unit u = j*128 + p). Then h = relu(ggA + b1 + ggG) on DVE/Scalar,
    [150]x[17] TensorE contraction, bias, DMA scores^T out.
  - Host scatters per-core outputs back to the original span order.

Compute dtype fp16 (rel err ~1e-3 vs f32 reference), f32 output.
"""

import os

os.environ.setdefault("MYCRO_LOCAL_CACHE", "1")

import numpy as np

# ---------------- problem constants (hardcoded per spec) ----------------
B, T, D = 32, 512, 1024
N_SPANS = 65536
H, L = 150, 17
HP = 256                 # table row elems (fp16 -> 512B, SWDGE granule)
NCORES = 8
BPC = B // NCORES        # batches per core = 4
TC = BPC * T             # tokens per core = 2048
N_KB = D // 128          # K blocks in stage 1 = 8
N_TT = TC // 128         # token tiles in stage 1 = 16
N_CH = 4                 # xsT load chunks
TPC = TC // N_CH         # tokens per load chunk = 512
SPAN_TILE = 512          # spans per stage-2 compute tile
GCH = int(os.environ.get("KGCH", "448"))   # spans per gather chunk
# ucode limit: transpose-gather RX descs/DMA = nidx*(elem/256B)/16 + 2
# must be <= 128 -> nidx <= 1008 -> GCH <= 504 spans (2 rows/span)
TAB_MODE = os.environ.get("KTAB", "sbuf")  # gather table: sbuf | dram
W1N = 2 * H              # 300: stage-1 moving operand width (A | G)


def build_graph(m_pad: int):
    """Build the per-core SPMD Bass graph. m_pad: padded span count."""
    from concourse import bacc
    import concourse.mybir as mybir
    from concourse.tile import TileContext

    fp16 = mybir.dt.float16
    f32 = mybir.dt.float32
    i16 = mybir.dt.int16
    AF = mybir.ActivationFunctionType
    ADD = mybir.AluOpType.add

    n_st = m_pad // SPAN_TILE
    n_gc = (m_pad + GCH - 1) // GCH   # gather chunks

    nc = bacc.Bacc(num_swdge_queues=4)

    xsT_d = nc.declare_dram_parameter("xsT", [128, N_CH * N_KB * TPC], fp16, isOutput=False)
    wc_d = nc.declare_dram_parameter("wc", [128, N_KB * W1N], fp16, isOutput=False)
    w2p_d = nc.declare_dram_parameter("w2p", [128, 2 * L], fp16, isOutput=False)
    b1p_d = nc.declare_dram_parameter("b1p", [128, 2], f32, isOutput=False)
    b2p_d = nc.declare_dram_parameter("b2p", [L, 1], f32, isOutput=False)
    idx_d = nc.declare_dram_parameter("idx", [128, 2 * m_pad // 16], i16, isOutput=False)
    outT_d = nc.declare_dram_parameter("outT", [L, m_pad], f32, isOutput=True)

    from concourse import library_config
    from concourse.tile_rust import add_dep_helper

    with TileContext(nc) as tc:
        with (
            tc.tile_pool(name="const", bufs=1) as constp,
            tc.tile_pool(name="xst", bufs=1) as xstp,
            tc.tile_pool(name="tab", bufs=1) as tabp,
            tc.tile_pool(name="tabt", bufs=4) as tabtp,
            tc.tile_pool(name="dram", bufs=1, space="DRAM") as dramp,
            tc.tile_pool(name="ps1", bufs=2, space="PSUM") as ps1p,
            tc.tile_pool(name="gat", bufs=3) as gatp,
            tc.tile_pool(name="act", bufs=8) as actp,
            tc.tile_pool(name="ps2", bufs=2, space="PSUM") as ps2p,
            tc.tile_pool(name="ot", bufs=4) as otp,
        ):
            nc.gpsimd.load_library(library_config.mlp)
            # ---- input loads: first xsT chunk + weights first so the
            # stage-1 matmuls can start immediately ----
            xst_sb = xstp.tile([128, N_CH, N_KB, TPC], fp16)
            nc.sync.dma_start(
                out=xst_sb[:, 0, :, :],
                in_=xsT_d[:, 0:N_KB * TPC],
            )
            wc_sb = constp.tile([128, N_KB * W1N], fp16)
            nc.scalar.dma_start(out=wc_sb[:], in_=wc_d[:])
            for c in range(1, N_CH):
                nc.sync.dma_start(
                    out=xst_sb[:, c, :, :],
                    in_=xsT_d[:, c * N_KB * TPC:(c + 1) * N_KB * TPC],
                )
            idx_sb = constp.tile([128, 2 * m_pad // 16], i16)
            nc.scalar.dma_start(out=idx_sb[:], in_=idx_d[:])
            w2p_sb = constp.tile([128, 2 * L], fp16)
            nc.scalar.dma_start(out=w2p_sb[:], in_=w2p_d[:])
            b1p_sb = constp.tile([128, 2], f32)
            nc.scalar.dma_start(out=b1p_sb[:], in_=b1p_d[:])
            b2p_sb = constp.tile([L, 1], f32)
            nc.scalar.dma_start(out=b2p_sb[:], in_=b2p_d[:])

            # A|G table: stripe tt = A rows of token tile tt, stripe
            # 16+tt = G rows. Row pad [H:HP) zeroed (gather reads whole
            # 512B rows; pad lands on unread partitions).
            if TAB_MODE == "sbuf":
                tab = tabp.tile([128, 2 * N_TT, HP], fp16)
                nc.vector.memset(tab[:, :, H:HP], 0.0)
            else:
                tab_d = dramp.tile([2 * TC, HP], fp16)
                tab_dmas = []

            # ---- stage 1: per token tile, A|G rows -> table stripes ----
            for tt in range(N_TT):
                ps = ps1p.tile([128, W1N], f32)
                c, t0 = tt // (N_TT // N_CH), (tt % (N_TT // N_CH)) * 128
                for kb in range(N_KB):
                    nc.tensor.matmul(
                        ps[:],
                        xst_sb[:, c, kb, t0:t0 + 128],
                        wc_sb[:, kb * W1N:(kb + 1) * W1N],
                        start=(kb == 0),
                        stop=(kb == N_KB - 1),
                    )
                if TAB_MODE == "sbuf":
                    nc.scalar.activation(tab[:, tt, 0:H], ps[:, 0:H], AF.Copy)
                    nc.scalar.activation(
                        tab[:, N_TT + tt, 0:H], ps[:, H:W1N], AF.Copy
                    )
                else:
                    tg = tabtp.tile([128, 2, HP], fp16, tag="tg")
                    nc.vector.memset(tg[:, :, H:HP], 0.0)
                    nc.scalar.activation(tg[:, 0, 0:H], ps[:, 0:H], AF.Copy)
                    nc.scalar.activation(tg[:, 1, 0:H], ps[:, H:W1N], AF.Copy)
                    tab_dmas.append(nc.sync.dma_start(
                        out=tab_d[tt * 128:(tt + 1) * 128, :], in_=tg[:, 0, :]
                    ))
                    tab_dmas.append(nc.sync.dma_start(
                        out=tab_d[TC + tt * 128:TC + (tt + 1) * 128, :],
                        in_=tg[:, 1, :],
                    ))

            # ---- stage 2: per gather chunk of GCH spans ----
            for g in range(n_gc):
                base = g * GCH
                nsp = min(GCH, m_pad - base)     # spans in this chunk
                nidx = 2 * nsp                   # A rows then G rows
                gg = gatp.tile([128, 2, nidx], fp16, tag=f"gg{nidx}")
                if TAB_MODE == "sbuf":
                    g_inst = nc.gpsimd.dma_gather(
                        gg[:],
                        tab[:],
                        idx_sb[:, 2 * base // 16:(2 * base + nidx) // 16],
                        nidx,
                        nidx,
                        elem_size=HP,
                        transpose=True,
                        queue_num=g % 4,
                        sbuf_tokens_per_rank=128,
                        sbuf_free_dim_per_rank=2 * HP,
                    )
                else:
                    g_inst = nc.gpsimd.dma_gather(
                        gg[:],
                        tab_d[:, :],
                        idx_sb[:, 2 * base // 16:(2 * base + nidx) // 16],
                        nidx,
                        nidx,
                        elem_size=HP,
                        transpose=True,
                        queue_num=g % 4,
                    )
                    for td in tab_dmas:
                        add_dep_helper(
                            g_inst.ins, td.ins, True, "gather after table"
                        )
                stile = min(SPAN_TILE, nsp)
                for j in range(nsp // stile):
                    s = j * stile
                    t0 = actp.tile([128, stile], fp16, tag=f"t0_{stile}")
                    t1 = actp.tile([22, stile], fp16, tag=f"t1_{stile}")
                    nc.vector.scalar_tensor_tensor(
                        t0[:], gg[:, 0, s:s + stile], b1p_sb[:, 0:1],
                        gg[:, 0, nsp + s:nsp + s + stile], ADD, ADD,
                    )
                    nc.vector.scalar_tensor_tensor(
                        t1[:], gg[0:22, 1, s:s + stile], b1p_sb[0:22, 1:2],
                        gg[0:22, 1, nsp + s:nsp + s + stile], ADD, ADD,
                    )
                    h0 = actp.tile([128, stile], fp16, tag=f"h0_{stile}")
                    h1 = actp.tile([22, stile], fp16, tag=f"h1_{stile}")
                    nc.scalar.activation(h0[:], t0[:], AF.Relu)
                    nc.vector.tensor_relu(h1[:], t1[:])
                    ps2 = ps2p.tile([L, stile], f32, tag=f"ps2_{stile}")
                    nc.tensor.matmul(
                        ps2[:], w2p_sb[:, 0:L], h0[:], start=True, stop=False
                    )
                    nc.tensor.matmul(
                        ps2[:], w2p_sb[0:22, L:2 * L], h1[:], start=False, stop=True
                    )
                    ot = otp.tile([L, stile], f32, tag=f"ot_{stile}")
                    nc.scalar.activation(ot[:], ps2[:], AF.Identity, bias=b2p_sb[:])
                    nc.sync.dma_start(
                        out=outT_d[:, base + s:base + s + stile], in_=ot[:]
                    )

    return nc


def prep_inputs(xs, spans, batch_ids, W1, b1, W2, b2):
    """Host-side routing and layout. Returns (in_maps, span_ids, m_pad)."""
    xs = np.asarray(xs, dtype=np.float32)
    spans = np.asarray(spans).astype(np.int64)
    batch_ids = np.asarray(batch_ids).astype(np.int64)
    W1 = np.asarray(W1, dtype=np.float32)
    b1 = np.asarray(b1, dtype=np.float32)
    W2 = np.asarray(W2, dtype=np.float32)
    b2 = np.asarray(b2, dtype=np.float32)

    core = batch_ids // BPC
    local0 = (batch_ids % BPC) * T + spans[:, 0]
    local1 = (batch_ids % BPC) * T + spans[:, 1]

    order = np.argsort(core, kind="stable")
    counts = np.bincount(core, minlength=NCORES)
    offs = np.concatenate([[0], np.cumsum(counts)])
    m_pad = int(max(np.ceil(counts.max() / SPAN_TILE), 1) * SPAN_TILE)

    # shared weights
    W1h = W1.astype(np.float16)
    wc = np.empty((128, N_KB * W1N), np.float16)
    for kb in range(N_KB):
        wc[:, kb * W1N:kb * W1N + H] = W1h[kb * 128:(kb + 1) * 128, :]
        wc[:, kb * W1N + H:(kb + 1) * W1N] = W1h[D + kb * 128:D + (kb + 1) * 128, :]
    W2pad = np.zeros((HP, L), np.float16)
    W2pad[:H] = W2.astype(np.float16)
    w2p = np.empty((128, 2 * L), np.float16)
    w2p[:, 0:L] = W2pad[0:128]
    w2p[:, L:2 * L] = W2pad[128:HP]
    b1pad = np.zeros((HP,), np.float32)
    b1pad[:H] = b1
    b1p = np.ascontiguousarray(b1pad.reshape(2, 128).T)
    b2p = np.ascontiguousarray(b2.reshape(L, 1))

    in_maps = []
    span_ids = []
    for c in range(NCORES):
        sel = order[offs[c]:offs[c + 1]]
        span_ids.append(sel)
        ncnt = len(sel)
        i0 = np.zeros(m_pad, np.int64)
        i1 = np.zeros(m_pad, np.int64)
        i0[:ncnt] = local0[sel]
        i1[:ncnt] = local1[sel]

        # gather index stream: per chunk of GCH spans, A rows (i0) then
        # G rows (TC+i1); 16-partition wrap, replicated x8 down partitions
        idx_cols = []
        for g in range(0, m_pad, GCH):
            nsp = min(GCH, m_pad - g)
            v = np.concatenate([i0[g:g + nsp], TC + i1[g:g + nsp]])
            idx_cols.append(v.reshape(-1, 16).T)
        idxc = np.tile(np.hstack(idx_cols), (8, 1)).astype(np.int16)

        # xsT packed [128, chunk, kb, token]: partition p, chunk ch,
        # block kb, token t -> xs_core[ch*TPC + t, kb*128 + p]
        xs_c = xs[c * BPC:(c + 1) * BPC].reshape(TC, D).astype(np.float16)
        xq = np.ascontiguousarray(
            xs_c.reshape(N_CH, TPC, N_KB, 128).transpose(3, 0, 2, 1)
        ).reshape(128, N_CH * N_KB * TPC)
        in_maps.append({
            "xsT": xq, "wc": wc, "w2p": w2p, "b1p": b1p, "b2p": b2p,
            "idx": np.ascontiguousarray(idxc),
        })

    return in_maps, span_ids, m_pad


def _scatter_out(results, span_ids):
    out = np.empty((N_SPANS, L), np.float32)
    for c in range(NCORES):
        sel = span_ids[c]
        out[sel] = results[c]["outT"].T[:len(sel)]
    return out


def _install_ntff_shim():
    """Provide antenv.axon_hooks (missing on this image) so that
    run_bass_kernel_spmd(trace=True) can drive NTFF profiling via the
    axon .so. Only used by the profiling path."""
    import sys
    import types
    import ctypes
    import contextlib

    if "antenv.axon_hooks" in sys.modules:
        return
    import antenv

    holder = {"hook": None}
    mod = types.ModuleType("antenv.axon_hooks")
    mod.set_axon_ntff_profile_hook = lambda h: holder.__setitem__("hook", h)
    mod.get_axon_ntff_profile_hook = lambda: holder["hook"]
    sys.modules["antenv.axon_hooks"] = mod
    antenv.axon_hooks = mod

    so_path = "/opt/axon/libaxon_pjrt.so"
    try:
        lib = ctypes.CDLL(so_path)
    except OSError:
        return
    if not hasattr(lib, "axon_start_nrt_profile"):
        return
    lib.axon_start_nrt_profile.argtypes = [
        ctypes.POINTER(ctypes.c_int64),
        ctypes.c_size_t,
    ]
    lib.axon_start_nrt_profile.restype = ctypes.c_int64
    lib.axon_stop_nrt_profile.argtypes = [ctypes.c_char_p]
    lib.axon_stop_nrt_profile.restype = ctypes.c_int64

    @contextlib.contextmanager
    def _hook(output_dir, device_ids):
        import jax

        jax.devices()
        if device_ids:
            ids = (ctypes.c_int64 * len(device_ids))(*device_ids)
            rc = lib.axon_start_nrt_profile(ids, len(device_ids))
        else:
            rc = lib.axon_start_nrt_profile(None, 0)
        if rc != 0:
            raise RuntimeError(f"axon_start_nrt_profile rc={rc}")
        try:
            yield
        finally:
            n = lib.axon_stop_nrt_profile(str(output_dir).encode())
            print(f"profile: {n} file(s) written to {output_dir}")

    mod.set_axon_ntff_profile_hook(_hook)


def run(inputs: dict, trace: bool = False):
    """Run on the 8 NeuronCores. Returns (out, BassKernelResults)."""
    from concourse import bass_utils
    from concourse.bass_utils import run_bass_kernel_spmd

    if trace:
        _install_ntff_shim()
        bass_utils.upload_artifacts = lambda tmpdir: str(tmpdir)

    in_maps, span_ids, m_pad = prep_inputs(**inputs)
    nc = build_graph(m_pad)
    nc.finalize()
    res = run_bass_kernel_spmd(
        nc, in_maps, list(range(NCORES)), trace=trace
    )
    return _scatter_out(res.results, span_ids), res


def kernel(**inputs) -> np.ndarray:
    out, _ = run(inputs, trace=False)
    return out
